# revision 1
# baseline (speedup 1.0000x reference)
"""Multi-head attention (projections + causal/padded softmax attention + output
projection + residual + LayerNorm) as a Bass/Tile kernel on 8 Trainium2 cores.

Sharding: tensor-parallel over heads within each batch. Core c handles batch
b = c // 4 and heads [4*(c%4), 4*(c%4)+4). Each core projects Q/K/V for its
4 heads over the full sequence, runs causal attention in a transposed layout
(scoresT[key, row]), and produces ctxT[dh, row]. Two 8-way AllToAlls (one per
head-pair, so the first overlaps the second pair's attention) redistribute
ctxT so core c ends with the full context dims for its 512-row quarter, on
which it runs the output projection, residual add and LayerNorm.

Layout trick: all matmul operands are pre-transposed/pre-cast on the host
(numpy) so every DMA is contiguous: qT/kT/vT = x^T as bf16, WqT/WkT/WvT/WoT =
W^T as bf16. The PE contracts over partitions, so the contraction dim (d_model
or d_head) always sits on the partition axis.

Softmax: scores are bounded (|s| ~ 5) so exp is computed without max
subtraction; exp(scale*s + pad_bias) runs on the scalar engine with the
padding mask folded into the per-key bias. The causal boundary is enforced by
zeroing probs with gpsimd.affine_select. The denominator is obtained by
augmenting V with a ones column (row 64 of ctxT psum = sum of probs); the
divide happens as broadcast+reciprocal+multiply on 64 partitions.

PSUM budget (8 banks): pj=3 (projection accumulators, reused as two Wo
halves in P3), sc=3 (score chunks, both heads round-robin), ctx=2.
"""

import math
from contextlib import ExitStack

import numpy as np
import ml_dtypes

import concourse.bass as bass
import concourse.mybir as mybir
import concourse.tile as tile
from concourse import bacc
from concourse.bass import ds
from concourse.bass_utils import run_bass_kernel_spmd

BF16 = mybir.dt.bfloat16
F32 = mybir.dt.float32

NEG_INF = -1e9
LN_EPS = 1e-6


class Cfg:
    def __init__(self, B=2, S=2048, D=1024, H=16, dh=64, kmax=None):
        self.B, self.S, self.D, self.H, self.dh = B, S, D, H, dh
        # kmax: max(sen_len) — keys beyond are fully masked, so K/V
        # projection and the attention key loop stop at this bound.
        self.kmax = S if kmax is None else min(int(kmax), S)
        self.NC = 8                      # cores
        self.G = 4                       # cores per batch group
        self.HPC = H // self.G           # heads per core
        self.PAIRS = self.HPC // 2       # head pairs per core
        self.D4 = self.HPC * dh          # per-core projection width
        self.RQ = S // self.G            # rows per core in Wo/LN phase
        self.NR = 4                      # attention row ranges
        self.RNG = S // self.NR          # rows per range (== RQ)
        self.DC = D // 128               # contraction chunks
        self.KCH = S // 128              # key chunks
        self.NS = max(1, S // 512)       # projection n-slices
        self.NSW = S // self.NS          # cols per n-slice
        self.WON = max(1, D // 512)      # Wo n-slices
        self.WONW = D // self.WON
        self.D4C = self.D4 // 128        # 128-chunks in per-core ctx width
        self.KB_MAX = -(-self.kmax // 128)          # key chunks actually used
        self.NS_K = -(-(self.KB_MAX * 128) // self.NSW)  # K-proj n-slices
        assert self.RQ == self.RNG
        assert self.PAIRS >= 1 and self.HPC % 2 == 0


def build_program(cfg: Cfg, debug_taps: bool = False):
    """Build the (SPMD-identical) Bass program."""
    nc = bacc.Bacc("TRN2", target_bir_lowering=False, debug=False,
                   num_devices=cfg.NC)

    S, D, dh = cfg.S, cfg.D, cfg.dh
    D4, RQ, RNG = cfg.D4, cfg.RQ, cfg.RNG

    qT = nc.dram_tensor("qT", [D, S], BF16, kind="ExternalInput").ap()
    kT = nc.dram_tensor("kT", [D, S], BF16, kind="ExternalInput").ap()
    vT = nc.dram_tensor("vT", [D, S], BF16, kind="ExternalInput").ap()
    wqT = nc.dram_tensor("wqT", [D, D4], BF16, kind="ExternalInput").ap()
    wkT = nc.dram_tensor("wkT", [D, D4], BF16, kind="ExternalInput").ap()
    wvT = nc.dram_tensor("wvT", [D, D4], BF16, kind="ExternalInput").ap()
    woT = nc.dram_tensor("woT", [D, D], BF16, kind="ExternalInput").ap()
    resid = nc.dram_tensor("resid", [RQ, D], F32, kind="ExternalInput").ap()
    pad_bias = nc.dram_tensor("pad_bias", [cfg.KCH, 128], F32,
                              kind="ExternalInput").ap()
    gamma = nc.dram_tensor("gamma", [1, D], F32, kind="ExternalInput").ap()
    beta = nc.dram_tensor("beta", [1, D], F32, kind="ExternalInput").ap()
    out_shard = nc.dram_tensor("out_shard", [RQ, D], F32,
                               kind="ExternalOutput").ap()
    if debug_taps:
        dbg_khT = nc.dram_tensor("dbg_khT", [128, cfg.PAIRS, S], BF16,
                                 kind="ExternalOutput").ap()
        dbg_qhT = nc.dram_tensor("dbg_qhT", [128, cfg.PAIRS, S], BF16,
                                 kind="ExternalOutput").ap()
        dbg_vh = nc.dram_tensor("dbg_vh", [128, cfg.KCH,
                                           cfg.HPC * (dh + 1)], BF16,
                                kind="ExternalOutput").ap()
        dbg_a2ain = nc.dram_tensor("dbg_a2ain", [cfg.NC, 128, RQ], BF16,
                                   kind="ExternalOutput").ap()
        dbg_a2aout = nc.dram_tensor("dbg_a2aout", [cfg.NC, 128, RQ], BF16,
                                    kind="ExternalOutput").ap()
        dbg_rbc = nc.dram_tensor("dbg_rbc", [128, RNG], F32,
                                 kind="ExternalOutput").ap()
        dbg_sc = nc.dram_tensor("dbg_sc", [128, RNG], F32,
                                kind="ExternalOutput").ap()
        dbg_probs = nc.dram_tensor("dbg_probs", [128, RNG], BF16,
                                   kind="ExternalOutput").ap()
        dbg_probs6 = nc.dram_tensor("dbg_probs6", [128, RNG], BF16,
                                    kind="ExternalOutput").ap()
        dbg_ctx = nc.dram_tensor("dbg_ctx", [dh + 1, RNG], F32,
                                 kind="ExternalOutput").ap()
        dbg_dbc = nc.dram_tensor("dbg_dbc", [128, RNG], F32,
                                 kind="ExternalOutput").ap()

    with tile.TileContext(nc) as tc, ExitStack() as ctx:
        consts = ctx.enter_context(tc.tile_pool(name="consts", bufs=1))
        xin = ctx.enter_context(tc.tile_pool(name="xin", bufs=2))
        proj = ctx.enter_context(tc.tile_pool(name="proj", bufs=1))
        att = ctx.enter_context(tc.tile_pool(name="att", bufs=4))
        small = ctx.enter_context(tc.tile_pool(name="small", bufs=4))
        lnp = ctx.enter_context(tc.tile_pool(name="lnp", bufs=2))
        ctxf = ctx.enter_context(tc.tile_pool(name="ctxf", bufs=1))
        dram = ctx.enter_context(
            tc.tile_pool(name="dram", bufs=1, space="DRAM"))
        psum = ctx.enter_context(
            tc.tile_pool(name="psum", bufs=1, space="PSUM"))

        # ---- prologue: constants (wo/gamma/beta deferred to P3) ------------
        wq_sb = consts.tile([128, cfg.DC, D4], BF16)
        wk_sb = consts.tile([128, cfg.DC, D4], BF16)
        wv_sb = consts.tile([128, cfg.DC, D4], BF16)
        for w_sb, w_dram in ((wk_sb, wkT), (wv_sb, wvT), (wq_sb, wqT)):
            nc.sync.dma_start(
                out=w_sb, in_=w_dram.rearrange("(c p) o -> p c o", p=128))

        pb_sb = consts.tile([128, cfg.KCH], F32)
        nc.sync.dma_start(out=pb_sb, in_=pad_bias.rearrange("c p -> p c"))

        # batch predicates: core c belongs to batch c // G. All A2A
        # staging/output DMAs use static addresses predicated on these, so
        # Tile tracks the dependencies exactly (dynamic register offsets
        # proved unreliable to order against the collective on HW).
        pid = nc.gpsimd.partition_id()
        blk4 = nc.gpsimd.scalar_reg_alu(mybir.AluOpType.bitwise_and, pid,
                                        cfg.G)
        blk = blk4

        a2a_in = dram.tile([cfg.NC, cfg.PAIRS, 128, RQ], BF16,
                           name="a2a_in")
        a2a_out = dram.tile([cfg.NC, cfg.PAIRS, 128, RQ], BF16,
                            name="a2a_out")

        # ---- P1: projections (K, V first so attention can start early) ----
        qhT_sb = proj.tile([128, cfg.PAIRS, S], BF16)
        khT_sb = proj.tile([128, cfg.PAIRS, S], BF16)
        vh_sb = proj.tile([128, cfg.KCH, cfg.HPC * (dh + 1)], BF16)

        def qk_proj(x_dram, w_sb, out_sb, ns_count=None):
            for ns in range(ns_count if ns_count is not None else cfg.NS):
                x_ns = xin.tile([128, cfg.DC, cfg.NSW], BF16, tag="x_ns",
                                name="x_ns")
                nc.sync.dma_start(
                    out=x_ns, in_=x_dram.rearrange("(c p) s -> p c s", p=128)
                    [:, :, ns * cfg.NSW:(ns + 1) * cfg.NSW])
                for pair in range(cfg.PAIRS):
                    ps = psum.tile([128, cfg.NSW], F32, tag="pj", bufs=3,
                                   name="ps_pj")
                    for dc in range(cfg.DC):
                        nc.tensor.matmul(
                            ps, w_sb[:, dc, pair * 128:(pair + 1) * 128],
                            x_ns[:, dc, :],
                            start=dc == 0, stop=dc == cfg.DC - 1)
                    nc.vector.tensor_copy(
                        out=out_sb[:, pair, ns * cfg.NSW:(ns + 1) * cfg.NSW],
                        in_=ps)

        qk_proj(kT, wk_sb, khT_sb, ns_count=cfg.NS_K)

        for kb in range(cfg.KB_MAX):
            v_kb = xin.tile([128, cfg.DC, 128], BF16, tag="v_kb")
            nc.sync.dma_start(
                out=v_kb, in_=vT.rearrange("(c p) s -> p c s", p=128)
                [:, :, kb * 128:(kb + 1) * 128])
            psv = psum.tile([128, D4], F32, tag="pj", bufs=3, name="ps_v")
            for dc in range(cfg.DC):
                nc.tensor.matmul(psv, v_kb[:, dc, :], wv_sb[:, dc, :],
                                 start=dc == 0, stop=dc == cfg.DC - 1)
            nc.vector.tensor_copy(
                out=vh_sb[:, kb, :].rearrange("p (h e) -> p h e", e=dh + 1)
                [:, :, 0:dh],
                in_=psv.rearrange("p (h e) -> p h e", e=dh))
            nc.vector.memset(
                vh_sb[:, kb, :].rearrange("p (h e) -> p h e", e=dh + 1)
                [:, :, dh:dh + 1], 1.0)

        qk_proj(qT, wq_sb, qhT_sb)

        # ---- P2: attention; per-pair A2A issued as soon as pair finishes ---
        ccb = {}
        for pair in range(cfg.PAIRS):
            for r in range(cfg.NR):
                nch = min(((r + 1) * RNG) // 128, cfg.KB_MAX)
                ctx_ps = [psum.tile([dh + 1, RNG], F32, tag=f"ctx{h2}",
                                    bufs=1, name=f"ctx_ps{h2}")
                          for h2 in range(2)]
                for kb in range(nch):
                    # causal column truncation: rows r*RNG+f with f < f0
                    # (= kb*128 - r*RNG) are entirely below the diagonal.
                    f0 = max(0, kb * 128 - r * RNG)
                    w = RNG - f0
                    sc = [psum.tile([128, RNG], F32, tag="sc", bufs=3,
                                    name=f"sc{h2}") for h2 in range(2)]
                    probs = [att.tile([128, RNG], BF16, tag=f"pr{h2}",
                                      name=f"probs{h2}") for h2 in range(2)]
                    for h2 in range(2):
                        lo, hi = 64 * h2, 64 * h2 + 64
                        nc.tensor.matmul(
                            sc[h2][:, 0:w],
                            khT_sb[lo:hi, pair, kb * 128:(kb + 1) * 128],
                            qhT_sb[lo:hi, pair,
                                   r * RNG + f0:(r + 1) * RNG],
                            start=True, stop=True)
                        nc.scalar.activation(
                            out=probs[h2][:, f0:], in_=sc[h2][:, 0:w],
                            func=mybir.ActivationFunctionType.Exp,
                            bias=pb_sb[:, kb:kb + 1],
                            scale=1.0 / math.sqrt(dh))
                        if f0 > 0 or kb * 128 == r * RNG:
                            # partial band: keep f - f0 >= p
                            nc.gpsimd.affine_select(
                                out=probs[h2][:, f0:f0 + 128],
                                in_=probs[h2][:, f0:f0 + 128],
                                pattern=[[1, 128]],
                                base=0,
                                channel_multiplier=-1,
                                compare_op=mybir.AluOpType.is_ge,
                                fill=0.0)
                        h = 2 * pair + h2
                        nc.tensor.matmul(
                            ctx_ps[h2][:, f0:],
                            vh_sb[:, kb, h * (dh + 1):(h + 1) * (dh + 1)],
                            probs[h2][:, f0:],
                            start=kb == 0, stop=kb == nch - 1)
                # epilogue: divide by denominator (row dh of ctx psum).
                # Pool can't read PSUM, so bounce the denom row via SBUF,
                # broadcast to 64 partitions, then reciprocal+mul run wide.
                stage = att.tile([128, RNG], BF16, tag="stage")
                for h2 in range(2):
                    den = small.tile([1, RNG], F32, tag="den", name="den")
                    nc.vector.tensor_copy(out=den,
                                          in_=ctx_ps[h2][dh:dh + 1, :])
                    dbc = small.tile([64, RNG], F32, tag="dbc", name="dbc")
                    nc.gpsimd.partition_broadcast(dbc, den)
                    rbc = small.tile([64, RNG], F32, tag="rbc", name="rbc")
                    nc.vector.reciprocal(rbc, dbc)
                    nc.vector.tensor_mul(
                        stage[64 * h2:64 * h2 + 64, :],
                        ctx_ps[h2][0:dh, :], rbc)
                nc.gpsimd.dma_start(
                    out=a2a_in[ds(blk + r, 1), pair, :, :], in_=stage)
        # hard barrier        # hard barrier: every staging DMA must have fully landed before the
        # collective reads a2a_in (observed stale-read corruption of the
        # last-staged shards without it)
        tc.strict_bb_all_engine_barrier()
        nc.gpsimd.collective_compute(
            "AllToAll", mybir.AluOpType.bypass,
            replica_groups=[list(range(cfg.NC))],
            ins=[a2a_in[:]], outs=[a2a_out[:]])
        tc.strict_bb_all_engine_barrier()
        for pair in range(cfg.PAIRS):
            for l in range(cfg.G):
                t_ccb = ctxf.tile([128, RQ], BF16, name=f"ccb_{pair}_{l}",
                                  tag=f"ccb_{pair}_{l}")
                nc.gpsimd.dma_start(
                    out=t_ccb, in_=a2a_out[ds(blk + l, 1), pair, :, :])
                ccb[(pair, l)] = t_ccb

        if debug_taps:
            nc.sync.dma_start(out=dbg_khT, in_=khT_sb)
            nc.sync.dma_start(out=dbg_qhT, in_=qhT_sb)
            nc.sync.dma_start(out=dbg_vh, in_=vh_sb)
            nc.gpsimd.dma_start(out=dbg_a2ain, in_=a2a_in[:, 0, :, :])
            nc.gpsimd.dma_start(out=dbg_a2aout, in_=a2a_out[:, 0, :, :])

        # ---- P3: Wo + residual + LayerNorm ---------------------------------
        wo_sb = consts.tile([128, cfg.DC, D], BF16)
        nc.sync.dma_start(out=wo_sb,
                          in_=woT.rearrange("(c p) o -> p c o", p=128))
        g_row = consts.tile([1, D], F32)
        b_row = consts.tile([1, D], F32)
        nc.sync.dma_start(out=g_row, in_=gamma)
        nc.sync.dma_start(out=b_row, in_=beta)
        gamma_bc = consts.tile([128, D], F32)
        beta_bc = consts.tile([128, D], F32)
        nc.gpsimd.partition_broadcast(gamma_bc, g_row)
        nc.gpsimd.partition_broadcast(beta_bc, b_row)
        eps_sb = consts.tile([128, 1], F32)
        nc.vector.memset(eps_sb, LN_EPS)

        n_jc = cfg.G * cfg.D4C  # total 128-chunks of context width D
        for t in range(RQ // 128):
            pso = [psum.tile([128, cfg.WONW], F32, tag="pj", bufs=3,
                             name=f"pso{nsl}") for nsl in range(cfg.WON)]
            for jc in range(n_jc):
                # global dh-chunk jc lives in a2a buffer of pair p, block l
                l, p = divmod(jc, cfg.PAIRS)
                cc = ccb[(p, l)][:, t * 128:(t + 1) * 128]
                for nsl in range(cfg.WON):
                    nc.tensor.matmul(
                        pso[nsl], cc,
                        wo_sb[:, jc, nsl * cfg.WONW:(nsl + 1) * cfg.WONW],
                        start=jc == 0, stop=jc == n_jc - 1)
            res = lnp.tile([128, D], F32, tag="res")
            nc.sync.dma_start(out=res, in_=resid[t * 128:(t + 1) * 128, :])
            x = lnp.tile([128, D], F32, tag="x")
            for nsl in range(cfg.WON):
                sl = slice(nsl * cfg.WONW, (nsl + 1) * cfg.WONW)
                nc.vector.tensor_add(x[:, sl], pso[nsl], res[:, sl])
            fmax = math.gcd(nc.vector.BN_STATS_FMAX, D)
            nsub = D // fmax
            stats = lnp.tile([128, nsub, nc.vector.BN_STATS_DIM], F32,
                             tag="stats")
            for sg in range(nsub):
                nc.vector.bn_stats(
                    out=stats[:, sg, :],
                    in_=x.rearrange("p (a b) -> p a b", a=nsub)[:, sg, :])
            mv = lnp.tile([128, nc.vector.BN_AGGR_DIM], F32, tag="mv")
            nc.vector.bn_aggr(out=mv, in_=stats)
            sd = lnp.tile([128, 1], F32, tag="sd")
            nc.scalar.activation(out=sd, in_=mv[:, 1:2],
                                 func=mybir.ActivationFunctionType.Sqrt,
                                 bias=eps_sb, scale=1.0)
            rstd = lnp.tile([128, 1], F32, tag="rstd")
            nc.vector.reciprocal(rstd, sd)
            y = lnp.tile([128, D], F32, tag="y")
            nc.vector.tensor_scalar(
                out=y, in0=x, scalar1=mv[:, 0:1], scalar2=rstd,
                op0=mybir.AluOpType.subtract, op1=mybir.AluOpType.mult)
            yg = lnp.tile([128, D], F32, tag="yg")
            nc.vector.tensor_mul(yg, y, gamma_bc)
            out_sb = lnp.tile([128, D], F32, tag="out_sb")
            nc.vector.tensor_add(out_sb, yg, beta_bc)
            nc.sync.dma_start(out=out_shard[t * 128:(t + 1) * 128, :],
                              in_=out_sb)

    nc.compile()
    return nc


def make_in_maps(cfg: Cfg, q, k, v, Wq, Wk, Wv, Wo, gamma, beta, sen_len):
    """Host-side sharding: slice/transpose/cast per core."""
    bf = ml_dtypes.bfloat16
    in_maps = []
    woT_full = np.ascontiguousarray(Wo.T.astype(bf))
    pos = np.arange(cfg.S)
    per_batch = {}
    for b in range(cfg.B):
        per_batch[b] = (
            np.ascontiguousarray(q[b].T.astype(bf)),
            np.ascontiguousarray(k[b].T.astype(bf)),
            np.ascontiguousarray(v[b].T.astype(bf)),
            np.where(pos < int(sen_len[b]), 0.0, NEG_INF).astype(np.float32),
        )
    for c in range(cfg.NC):
        b = c // cfg.G
        l = c % cfg.G
        hs = slice(l * cfg.D4, (l + 1) * cfg.D4)
        rows = slice(l * cfg.RQ, (l + 1) * cfg.RQ)
        qTb, kTb, vTb, pb = per_batch[b]
        in_maps.append({
            "qT": qTb, "kT": kTb, "vT": vTb,
            "wqT": np.ascontiguousarray(Wq[hs, :].T.astype(bf)),
            "wkT": np.ascontiguousarray(Wk[hs, :].T.astype(bf)),
            "wvT": np.ascontiguousarray(Wv[hs, :].T.astype(bf)),
            "woT": woT_full,
            "resid": np.ascontiguousarray(q[b, rows, :]).astype(np.float32),
            "pad_bias": pb.reshape(cfg.KCH, 128),
            "gamma": gamma.reshape(1, cfg.D).astype(np.float32),
            "beta": beta.reshape(1, cfg.D).astype(np.float32),
        })
    return in_maps


def assemble_output(cfg: Cfg, results):
    out = np.empty((cfg.B, cfg.S, cfg.D), np.float32)
    for c in range(cfg.NC):
        b, l = c // cfg.G, c % cfg.G
        out[b, l * cfg.RQ:(l + 1) * cfg.RQ, :] = results[c]["out_shard"]
    return out


_PROGRAM_CACHE = {}


def _get_program(cfg: Cfg):
    key = (cfg.B, cfg.S, cfg.D, cfg.H, cfg.dh, cfg.KB_MAX)
    if key not in _PROGRAM_CACHE:
        _PROGRAM_CACHE[key] = build_program(cfg)
    return _PROGRAM_CACHE[key]


def run(cfg: Cfg, inputs: dict, trace: bool = False):
    nc = _get_program(cfg)
    in_maps = make_in_maps(cfg, **inputs)
    res = run_bass_kernel_spmd(nc, in_maps, core_ids=list(range(cfg.NC)),
                               trace=trace)
    return assemble_output(cfg, res.results), res


def kernel(**inputs) -> np.ndarray:
    kmax = int(np.max(inputs["sen_len"]))
    cfg = Cfg(B=2, S=2048, D=1024, H=16, dh=64, kmax=kmax)
    out, _ = run(cfg, inputs)
    return out



# revision 13
# speedup vs baseline: 1.4575x; 1.4575x over previous
"""Multi-head attention (projections + causal/padded softmax attention + output
projection + residual + LayerNorm) as a Bass/Tile kernel on 8 Trainium2 cores.

Sharding: tensor-parallel over heads within each batch. Core c handles batch
b = c // 4 and heads [4*(c%4), 4*(c%4)+4). Each core projects Q/K/V for its
4 heads over the full sequence, runs causal attention in a transposed layout
(scoresT[key, row]), and produces ctxT[dh, row]. An 8-way AllToAll
redistributes ctxT so core c ends with the full context dims for its 512-row
quarter, on which it runs the output projection, residual add and LayerNorm.

Layout trick: all matmul operands are pre-transposed/pre-cast on the host
(numpy) so every DMA is contiguous: qT/kT/vT = x^T as bf16, WqT/WkT/WvT/WoT =
W^T as bf16. The PE contracts over partitions, so the contraction dim (d_model
or d_head) always sits on the partition axis.

P2 structure (v2): per (pair, row-range) the score matmuls for ALL key chunks
run first (64x128 row-tiled mode, the two heads of the pair in concurrent PE
row tiles writing the two banks of one [128, 2, 512] psum tile), each chunk
exp'd by ONE wide scalar activation (pad bias folded in) into SBUF probs.
The ctx matmuls (128-mode) for task i run after the scores of task i+1 have
been emitted, so the PE never ping-pongs tiling modes per chunk and the ctx
burst's wait-on-exp overlaps the next task's score/exp pipeline.

Softmax: scores are bounded (|s| ~ 5) so exp is computed without max
subtraction. Causal boundary enforced by zeroing probs with
gpsimd.affine_select. Denominator comes from a ones column appended to V
(row 64 of ctxT psum = sum of probs); division = reciprocal_approx_fast on
the [1,512] denom row, gpsimd partition_broadcast, one [64,512] multiply.

PSUM budget (8 banks): pj=2 (projection accumulators; reused as the 2-bank
Wo accumulator via the sc tag in P3), sc=2x2banks, ctx=2.
"""

import math
import os
from contextlib import ExitStack

import numpy as np
import ml_dtypes

# Bisection toggles (default = fast path). Set env to "0" to revert a piece.
WIDE_EXP = os.environ.get("K_WIDE_EXP", "1") == "1"
FAST_RECIP = os.environ.get("K_FAST_RECIP", "1") == "1"
P3_WIDE = os.environ.get("K_P3_WIDE", "1") == "1"
PIPELINED = os.environ.get("K_PIPELINED", "1") == "1"

import concourse.bass as bass
import concourse.mybir as mybir
import concourse.tile as tile
from concourse import bacc
from concourse.bass import ds
from concourse.bass_utils import run_bass_kernel_spmd

BF16 = mybir.dt.bfloat16
F32 = mybir.dt.float32

NEG_INF = -1e9
LN_EPS = 1e-6


class Cfg:
    def __init__(self, B=2, S=2048, D=1024, H=16, dh=64, kmax=None):
        self.B, self.S, self.D, self.H, self.dh = B, S, D, H, dh
        # kmax: max(sen_len) — keys beyond are fully masked, so K/V
        # projection and the attention key loop stop at this bound.
        self.kmax = S if kmax is None else min(int(kmax), S)
        self.NC = 8                      # cores
        self.G = 4                       # cores per batch group
        self.HPC = H // self.G           # heads per core
        self.PAIRS = self.HPC // 2       # head pairs per core
        self.D4 = self.HPC * dh          # per-core projection width
        self.RQ = S // self.G            # rows per core in Wo/LN phase
        self.NR = 4                      # attention row ranges
        self.RNG = S // self.NR          # rows per range (== RQ)
        self.DC = D // 128               # contraction chunks
        self.KCH = S // 128              # key chunks
        self.NS = max(1, S // 512)       # projection n-slices
        self.NSW = S // self.NS          # cols per n-slice
        self.WON = max(1, D // 512)      # Wo n-slices
        self.WONW = D // self.WON
        self.D4C = self.D4 // 128        # 128-chunks in per-core ctx width
        self.KB_MAX = -(-self.kmax // 128)          # key chunks actually used
        self.NS_K = -(-(self.KB_MAX * 128) // self.NSW)  # K-proj n-slices
        assert self.RQ == self.RNG
        assert self.PAIRS >= 1 and self.HPC % 2 == 0


def build_program(cfg: Cfg):
    """Build the (SPMD-identical) Bass program."""
    nc = bacc.Bacc("TRN2", target_bir_lowering=False, debug=False,
                   num_devices=cfg.NC)

    S, D, dh = cfg.S, cfg.D, cfg.dh
    D4, RQ, RNG = cfg.D4, cfg.RQ, cfg.RNG

    qT = nc.dram_tensor("qT", [D, S], BF16, kind="ExternalInput").ap()
    kT = nc.dram_tensor("kT", [D, S], BF16, kind="ExternalInput").ap()
    vT = nc.dram_tensor("vT", [D, S], BF16, kind="ExternalInput").ap()
    wqT = nc.dram_tensor("wqT", [D, D4], BF16, kind="ExternalInput").ap()
    wkT = nc.dram_tensor("wkT", [D, D4], BF16, kind="ExternalInput").ap()
    wvT = nc.dram_tensor("wvT", [D, D4], BF16, kind="ExternalInput").ap()
    woT = nc.dram_tensor("woT", [D, D], BF16, kind="ExternalInput").ap()
    resid = nc.dram_tensor("resid", [RQ, D], F32, kind="ExternalInput").ap()
    pad_bias = nc.dram_tensor("pad_bias", [cfg.KCH, 128], F32,
                              kind="ExternalInput").ap()
    gamma = nc.dram_tensor("gamma", [1, D], F32, kind="ExternalInput").ap()
    beta = nc.dram_tensor("beta", [1, D], F32, kind="ExternalInput").ap()
    out_shard = nc.dram_tensor("out_shard", [RQ, D], F32,
                               kind="ExternalOutput").ap()

    with tile.TileContext(nc) as tc, ExitStack() as ctx:
        consts = ctx.enter_context(tc.tile_pool(name="consts", bufs=1))
        xin = ctx.enter_context(tc.tile_pool(name="xin", bufs=2))
        proj = ctx.enter_context(tc.tile_pool(name="proj", bufs=1))
        att = ctx.enter_context(tc.tile_pool(name="att", bufs=4))
        small = ctx.enter_context(tc.tile_pool(name="small", bufs=4))
        lnp = ctx.enter_context(tc.tile_pool(name="lnp", bufs=2))
        dram = ctx.enter_context(
            tc.tile_pool(name="dram", bufs=1, space="DRAM"))
        psum = ctx.enter_context(
            tc.tile_pool(name="psum", bufs=1, space="PSUM"))

        # ---- prologue: constants (DMA order tuned: K path first) ----------
        wq_sb = consts.tile([128, cfg.DC, D4], BF16)
        wk_sb = consts.tile([128, cfg.DC, D4], BF16)
        wv_sb = consts.tile([128, cfg.DC, D4], BF16)
        pb_sb = consts.tile([128, cfg.KCH], F32)
        nc.sync.dma_start(
            out=wk_sb, in_=wkT.rearrange("(c p) o -> p c o", p=128))
        nc.sync.dma_start(out=pb_sb, in_=pad_bias.rearrange("c p -> p c"))

        # batch predicates: core c belongs to batch c // G. All A2A
        # staging/output DMAs use static addresses predicated on these, so
        # Tile tracks the dependencies exactly.
        pid = nc.gpsimd.partition_id()
        blk = nc.gpsimd.scalar_reg_alu(mybir.AluOpType.bitwise_and, pid,
                                       cfg.G)

        a2a_in = dram.tile([cfg.NC, cfg.PAIRS, 128, RQ], BF16,
                           name="a2a_in")
        a2a_out = dram.tile([cfg.NC, cfg.PAIRS, 128, RQ], BF16,
                            name="a2a_out")

        # ---- P1: projection helpers ---------------------------------------
        qhT_sb = proj.tile([128, cfg.PAIRS, S], BF16)
        khT_sb = proj.tile([128, cfg.PAIRS, S], BF16)
        vh_sb = proj.tile([128, cfg.KB_MAX, cfg.HPC * (dh + 1)], BF16)

        def qk_proj(x_dram, w_sb, out_sb, ns):
            x_ns = xin.tile([128, cfg.DC, cfg.NSW], BF16, tag="x_ns",
                            name="x_ns")
            nc.sync.dma_start(
                out=x_ns, in_=x_dram.rearrange("(c p) s -> p c s", p=128)
                [:, :, ns * cfg.NSW:(ns + 1) * cfg.NSW])
            for pair in range(cfg.PAIRS):
                ps = psum.tile([128, cfg.NSW], F32, tag="pj", bufs=2,
                               name="ps_pj")
                for dc in range(cfg.DC):
                    nc.tensor.matmul(
                        ps, w_sb[:, dc, pair * 128:(pair + 1) * 128],
                        x_ns[:, dc, :],
                        start=dc == 0, stop=dc == cfg.DC - 1)
                nc.vector.tensor_copy(
                    out=out_sb[:, pair, ns * cfg.NSW:(ns + 1) * cfg.NSW],
                    in_=ps)

        def v_proj(kb):
            v_kb = xin.tile([128, cfg.DC, 128], BF16, tag="v_kb")
            nc.sync.dma_start(
                out=v_kb, in_=vT.rearrange("(c p) s -> p c s", p=128)
                [:, :, kb * 128:(kb + 1) * 128])
            psv = psum.tile([128, D4], F32, tag="pj", bufs=2, name="ps_v")
            for dc in range(cfg.DC):
                nc.tensor.matmul(psv, v_kb[:, dc, :], wv_sb[:, dc, :],
                                 start=dc == 0, stop=dc == cfg.DC - 1)
            nc.vector.tensor_copy(
                out=vh_sb[:, kb, :].rearrange("p (h e) -> p h e", e=dh + 1)
                [:, :, 0:dh],
                in_=psv.rearrange("p (h e) -> p h e", e=dh))
            nc.vector.memset(
                vh_sb[:, kb, :].rearrange("p (h e) -> p h e", e=dh + 1)
                [:, :, dh:dh + 1], 1.0)

        wo_sb = consts.tile([128, cfg.DC, D], BF16)
        g_row = consts.tile([1, D], F32)
        b_row = consts.tile([1, D], F32)
        gamma_bc = consts.tile([128, D], F32)
        beta_bc = consts.tile([128, D], F32)
        eps_sb = consts.tile([128, 1], F32)
        res_sb = consts.tile([128, RQ // 128, D], F32)

        def emit_p3_consts():
            # P3 constants — emitted mid-P2 so their (4 MB of) DMAs land
            # during attention without delaying the P1 input loads.
            nc.sync.dma_start(out=wo_sb,
                              in_=woT.rearrange("(c p) o -> p c o", p=128))
            nc.sync.dma_start(out=g_row, in_=gamma)
            nc.sync.dma_start(out=b_row, in_=beta)
            nc.gpsimd.partition_broadcast(gamma_bc, g_row)
            nc.gpsimd.partition_broadcast(beta_bc, b_row)
            nc.vector.memset(eps_sb, LN_EPS)
            nc.sync.dma_start(out=res_sb,
                              in_=resid.rearrange("(t p) d -> p t d", p=128))

        # ---- P2: attention -------------------------------------------------
        # Per task (pair, r): emit score matmuls + exp for all key chunks
        # (64x128 row-tiled mode, both heads concurrent); the ctx matmul
        # burst (128-mode) for task i is emitted after the scores of task
        # i+1 so the PE keeps a stable mode per burst and the ctx wait on
        # the last exp overlaps the next task's pipeline.
        def nch_of(r):
            return min(((r + 1) * RNG) // 128, cfg.KB_MAX)

        def f0_of(r, kb):
            return max(0, kb * 128 - r * RNG)

        probs_map = {}

        def emit_scores(pair, r):
            nch = nch_of(r)
            for kb in range(nch):
                f0 = f0_of(r, kb)
                sc = psum.tile([128, 2, RNG], F32, tag="sc", bufs=2,
                               name="sc")
                for h2 in range(2):
                    lo, hi = 64 * h2, 64 * h2 + 64
                    nc.tensor.matmul(
                        sc[:, h2, f0:],
                        khT_sb[lo:hi, pair, kb * 128:(kb + 1) * 128],
                        qhT_sb[lo:hi, pair, r * RNG + f0:(r + 1) * RNG],
                        start=True, stop=True)
                probs = att.tile([128, 2, RNG], BF16, tag="pr", bufs=16,
                                 name="probs")
                if WIDE_EXP and f0 == 0:
                    # flat [128, 1024] over both psum banks in one ACT
                    nc.scalar.activation(
                        out=probs.rearrange("p a b -> p (a b)"),
                        in_=sc.rearrange("p a b -> p (a b)"),
                        func=mybir.ActivationFunctionType.Exp,
                        bias=pb_sb[:, kb:kb + 1],
                        scale=1.0 / math.sqrt(dh))
                else:
                    for h2 in range(2):
                        nc.scalar.activation(
                            out=probs[:, h2, f0:], in_=sc[:, h2, f0:],
                            func=mybir.ActivationFunctionType.Exp,
                            bias=pb_sb[:, kb:kb + 1],
                            scale=1.0 / math.sqrt(dh))
                if f0 > 0 or kb * 128 == r * RNG:
                    # partial band: keep f - f0 >= p
                    for h2 in range(2):
                        nc.gpsimd.affine_select(
                            out=probs[:, h2, f0:f0 + 128],
                            in_=probs[:, h2, f0:f0 + 128],
                            pattern=[[1, 128]],
                            base=0,
                            channel_multiplier=-1,
                            compare_op=mybir.AluOpType.is_ge,
                            fill=0.0)
                probs_map[(pair, r, kb)] = probs

        def emit_ctx(pair, r):
            nch = nch_of(r)
            ctx_ps = [psum.tile([dh + 1, RNG], F32, tag=f"ctx{h2}",
                                bufs=1, name=f"ctx_ps{h2}")
                      for h2 in range(2)]
            for kb in range(nch):
                f0 = f0_of(r, kb)
                probs = probs_map.pop((pair, r, kb))
                for h2 in range(2):
                    h = 2 * pair + h2
                    nc.tensor.matmul(
                        ctx_ps[h2][:, f0:],
                        vh_sb[:, kb, h * (dh + 1):(h + 1) * (dh + 1)],
                        probs[:, h2, f0:],
                        start=kb == 0, stop=kb == nch - 1)
            # epilogue: divide by the denominator (row dh of ctx psum).
            stage = att.tile([128, RNG], BF16, tag="stage", bufs=4)
            for h2 in range(2):
                den = small.tile([1, RNG], F32, tag="den", name="den")
                dcp = small.tile([1, RNG], F32, tag="dcp", name="dcp")
                nc.vector.tensor_copy(out=dcp,
                                      in_=ctx_ps[h2][dh:dh + 1, :])
                if FAST_RECIP:
                    nc.vector.reciprocal_approx_fast(out=den, in_=dcp)
                else:
                    nc.vector.reciprocal(den, dcp)
                dbc = small.tile([64, RNG], F32, tag="dbc", name="dbc")
                nc.gpsimd.partition_broadcast(dbc, den)
                nc.vector.tensor_mul(
                    stage[64 * h2:64 * h2 + 64, :],
                    ctx_ps[h2][0:dh, :], dbc)
            nc.gpsimd.dma_start(
                out=a2a_in[ds(blk + r, 1), pair, :, :], in_=stage)

        # Fused P1/P2 schedule: K/Q slice 0 project first, scores(0,0)
        # starts immediately after, and the remaining K/V/Q projection
        # slices are prefetched just-in-time between attention tasks so
        # the PE fills the exp-wait gaps and the input DMAs stream in
        # need-order.
        qk_proj(kT, wk_sb, khT_sb, 0)
        nc.sync.dma_start(
            out=wq_sb, in_=wqT.rearrange("(c p) o -> p c o", p=128))
        qk_proj(qT, wq_sb, qhT_sb, 0)
        nc.sync.dma_start(
            out=wv_sb, in_=wvT.rearrange("(c p) o -> p c o", p=128))

        tasks = [(p, r) for p in range(cfg.PAIRS) for r in range(cfg.NR)]
        k_done, v_done, q_done = 1, 0, 1
        for i, (pair, r) in enumerate(tasks):
            emit_scores(pair, r)
            if i + 1 < len(tasks):
                nr_ = tasks[i + 1][1]
                need_k = min(-(-(nch_of(nr_) * 128) // cfg.NSW), cfg.NS_K)
                while k_done < need_k:
                    qk_proj(kT, wk_sb, khT_sb, k_done)
                    k_done += 1
                while v_done < nch_of(nr_):
                    v_proj(v_done)
                    v_done += 1
                while q_done < min(nr_ + 1, cfg.NS):
                    qk_proj(qT, wq_sb, qhT_sb, q_done)
                    q_done += 1
            if i == min(3, len(tasks) - 1):
                emit_p3_consts()
            if PIPELINED:
                if i > 0:
                    emit_ctx(*tasks[i - 1])
            else:
                emit_ctx(pair, r)
        while v_done < nch_of(cfg.NR - 1):   # safety for degenerate shapes
            v_proj(v_done)
            v_done += 1
        if PIPELINED:
            emit_ctx(*tasks[-1])

        # hard barrier: every staging DMA must have fully landed before the
        # collective reads a2a_in (observed stale-read corruption of the
        # last-staged shards without it)
        tc.strict_bb_all_engine_barrier()
        nc.gpsimd.collective_compute(
            "AllToAll", mybir.AluOpType.bypass,
            replica_groups=[list(range(cfg.NC))],
            ins=[a2a_in[:]], outs=[a2a_out[:]])
        tc.strict_bb_all_engine_barrier()
        ccb = {}
        for pair in range(cfg.PAIRS):
            for l in range(cfg.G):
                t_ccb = att.tile([128, RQ], BF16, name=f"ccb_{pair}_{l}",
                                 tag=f"ccb_{pair}_{l}", bufs=1)
                nc.gpsimd.dma_start(
                    out=t_ccb, in_=a2a_out[ds(blk + l, 1), pair, :, :])
                ccb[(pair, l)] = t_ccb

        # ---- P3: Wo + residual + LayerNorm ---------------------------------
        n_jc = cfg.G * cfg.D4C  # total 128-chunks of context width D
        assert cfg.WON == 2 and cfg.WONW == RNG
        for t in range(RQ // 128):
            if P3_WIDE:
                pso_t = psum.tile([128, 2, cfg.WONW], F32, tag="sc", bufs=2,
                                  name="pso")
                pso = [pso_t[:, nsl, :] for nsl in range(cfg.WON)]
            else:
                pso = [psum.tile([128, cfg.WONW], F32, tag="pj", bufs=2,
                                 name=f"pso{nsl}") for nsl in range(cfg.WON)]
            for jc in range(n_jc):
                # global dh-chunk jc lives in a2a buffer of pair p, block l
                l, p = divmod(jc, cfg.PAIRS)
                cc = ccb[(p, l)][:, t * 128:(t + 1) * 128]
                for nsl in range(cfg.WON):
                    nc.tensor.matmul(
                        pso[nsl], cc,
                        wo_sb[:, jc, nsl * cfg.WONW:(nsl + 1) * cfg.WONW],
                        start=jc == 0, stop=jc == n_jc - 1)
            x = lnp.tile([128, D], F32, tag="x")
            if P3_WIDE:
                nc.vector.tensor_add(x, pso_t.rearrange("p a b -> p (a b)"),
                                     res_sb[:, t, :])
            else:
                for nsl in range(cfg.WON):
                    sl = slice(nsl * cfg.WONW, (nsl + 1) * cfg.WONW)
                    nc.vector.tensor_add(x[:, sl], pso[nsl],
                                         res_sb[:, t, sl])
            fmax = math.gcd(nc.vector.BN_STATS_FMAX, D)
            nsub = D // fmax
            stats = lnp.tile([128, nsub, nc.vector.BN_STATS_DIM], F32,
                             tag="stats")
            for sg in range(nsub):
                nc.vector.bn_stats(
                    out=stats[:, sg, :],
                    in_=x.rearrange("p (a b) -> p a b", a=nsub)[:, sg, :])
            mv = lnp.tile([128, nc.vector.BN_AGGR_DIM], F32, tag="mv")
            nc.vector.bn_aggr(out=mv, in_=stats)
            sd = lnp.tile([128, 1], F32, tag="sd")
            nc.scalar.activation(out=sd, in_=mv[:, 1:2],
                                 func=mybir.ActivationFunctionType.Sqrt,
                                 bias=eps_sb, scale=1.0)
            rstd = lnp.tile([128, 1], F32, tag="rstd")
            nc.vector.reciprocal(rstd, sd)
            y = lnp.tile([128, D], F32, tag="y")
            nc.vector.tensor_scalar(
                out=y, in0=x, scalar1=mv[:, 0:1], scalar2=rstd,
                op0=mybir.AluOpType.subtract, op1=mybir.AluOpType.mult)
            yg = lnp.tile([128, D], F32, tag="yg")
            nc.vector.tensor_mul(yg, y, gamma_bc)
            out_sb = lnp.tile([128, D], F32, tag="out_sb")
            nc.vector.tensor_add(out_sb, yg, beta_bc)
            nc.sync.dma_start(out=out_shard[t * 128:(t + 1) * 128, :],
                              in_=out_sb)

    nc.compile()
    return nc


def make_in_maps(cfg: Cfg, q, k, v, Wq, Wk, Wv, Wo, gamma, beta, sen_len):
    """Host-side sharding: slice/transpose/cast per core."""
    bf = ml_dtypes.bfloat16
    in_maps = []
    woT_full = np.ascontiguousarray(Wo.T.astype(bf))
    pos = np.arange(cfg.S)
    per_batch = {}
    for b in range(cfg.B):
        per_batch[b] = (
            np.ascontiguousarray(q[b].T.astype(bf)),
            np.ascontiguousarray(k[b].T.astype(bf)),
            np.ascontiguousarray(v[b].T.astype(bf)),
            np.where(pos < int(sen_len[b]), 0.0, NEG_INF).astype(np.float32),
        )
    for c in range(cfg.NC):
        b = c // cfg.G
        l = c % cfg.G
        hs = slice(l * cfg.D4, (l + 1) * cfg.D4)
        rows = slice(l * cfg.RQ, (l + 1) * cfg.RQ)
        qTb, kTb, vTb, pb = per_batch[b]
        in_maps.append({
            "qT": qTb, "kT": kTb, "vT": vTb,
            "wqT": np.ascontiguousarray(Wq[hs, :].T.astype(bf)),
            "wkT": np.ascontiguousarray(Wk[hs, :].T.astype(bf)),
            "wvT": np.ascontiguousarray(Wv[hs, :].T.astype(bf)),
            "woT": woT_full,
            "resid": np.ascontiguousarray(q[b, rows, :]).astype(np.float32),
            "pad_bias": pb.reshape(cfg.KCH, 128),
            "gamma": gamma.reshape(1, cfg.D).astype(np.float32),
            "beta": beta.reshape(1, cfg.D).astype(np.float32),
        })
    return in_maps


def assemble_output(cfg: Cfg, results):
    out = np.empty((cfg.B, cfg.S, cfg.D), np.float32)
    for c in range(cfg.NC):
        b, l = c // cfg.G, c % cfg.G
        out[b, l * cfg.RQ:(l + 1) * cfg.RQ, :] = results[c]["out_shard"]
    return out


_PROGRAM_CACHE = {}


def _get_program(cfg: Cfg):
    key = (cfg.B, cfg.S, cfg.D, cfg.H, cfg.dh, cfg.KB_MAX)
    if key not in _PROGRAM_CACHE:
        _PROGRAM_CACHE[key] = build_program(cfg)
    return _PROGRAM_CACHE[key]


def run(cfg: Cfg, inputs: dict, trace: bool = False):
    nc = _get_program(cfg)
    in_maps = make_in_maps(cfg, **inputs)
    res = run_bass_kernel_spmd(nc, in_maps, core_ids=list(range(cfg.NC)),
                               trace=trace)
    return assemble_output(cfg, res.results), res


def kernel(**inputs) -> np.ndarray:
    kmax = int(np.max(inputs["sen_len"]))
    cfg = Cfg(B=2, S=2048, D=1024, H=16, dh=64, kmax=kmax)
    out, _ = run(cfg, inputs)
    return out
